# revision 24
# baseline (speedup 1.0000x reference)
"""Trainium2 Bass kernel for multi-head attention (B=2, H=16, S=2048, hd=64, RoPE, causal).

Sharding: 8 cores = 2 (batch) x 4 (head groups of 4 heads).  Each core computes
its batch's Q/K/V projections for its 4 heads, RoPE, causal softmax attention,
and a partial out-projection.  Host gather sums the 4 partials per batch
(the out_proj all-reduce) and adds the bias.
"""

import math
import os
import sys
from contextlib import ExitStack

import numpy as np

sys.path.insert(0, "/opt/trn_rl_repo")

import ml_dtypes  # noqa: E402

import concourse.bass as bass  # noqa: E402
import concourse.bacc as bacc  # noqa: E402
import concourse.tile as tile  # noqa: E402
from concourse import mybir  # noqa: E402
from concourse.bass_utils import run_bass_kernel_spmd  # noqa: E402

BF16 = ml_dtypes.bfloat16
P = 128
S = 2048
D = 1024
HD = 64
E = 256  # head dims per core (4 heads x 64)
N_CORES = 8
SCALE = 1.0 / math.sqrt(HD)

bf = mybir.dt.bfloat16
f32 = mybir.dt.float32
f32r = mybir.dt.float32r
EXP = mybir.ActivationFunctionType.Exp

_CACHE = {}


def build_nc():
    nc = bacc.Bacc(None, target_bir_lowering=False, debug=False)

    xt_d = nc.declare_dram_parameter("xt", [D, S], bf, isOutput=False)
    wq_d = nc.declare_dram_parameter("wq_t", [D, E], bf, isOutput=False)
    wk_d = nc.declare_dram_parameter("wk_t", [D, E], bf, isOutput=False)
    wv_d = nc.declare_dram_parameter("wv_t", [D, E], bf, isOutput=False)
    wo_d = nc.declare_dram_parameter("wo_t", [E, D], bf, isOutput=False)
    cos_d = nc.declare_dram_parameter("cos2", [P, S], bf, isOutput=False)
    sin_d = nc.declare_dram_parameter("sinn2", [P, S], bf, isOutput=False)
    msk_d = nc.declare_dram_parameter("mask01", [P, P], bf, isOutput=False)
    out_d = nc.declare_dram_parameter("out_p", [S, D], f32, isOutput=True)

    with tile.TileContext(nc) as tc, ExitStack() as ctx:
        pers = ctx.enter_context(tc.tile_pool(name="pers", bufs=1))
        rtmp_pool = ctx.enter_context(tc.tile_pool(name="rtmp", bufs=2))
        ex_pool = ctx.enter_context(tc.tile_pool(name="ex", bufs=3))
        rc_pool = ctx.enter_context(tc.tile_pool(name="rc", bufs=2))
        rb_pool = ctx.enter_context(tc.tile_pool(name="rb", bufs=2))
        oev_pool = ctx.enter_context(tc.tile_pool(name="oev", bufs=4))
        ps512 = ctx.enter_context(tc.tile_pool(name="ps512", bufs=4, space="PSUM"))
        sc_ps = ctx.enter_context(tc.tile_pool(name="scps", bufs=1, space="PSUM"))

        xt = [pers.tile([P, S], bf, tag=f"xt{k}", name=f"xt{k}") for k in range(8)]
        wq = [pers.tile([P, E], bf, tag=f"wq{k}", name=f"wq{k}") for k in range(8)]
        wk = [pers.tile([P, E], bf, tag=f"wk{k}", name=f"wk{k}") for k in range(8)]
        wv = [pers.tile([P, E], bf, tag=f"wv{k}", name=f"wv{k}") for k in range(8)]
        wo = [pers.tile([P, D], bf, tag=f"wo{m}", name=f"wo{m}") for m in range(2)]
        cos2 = pers.tile([P, S], bf, tag="cos2", name="cos2")
        sinn2 = pers.tile([P, S], bf, tag="sinn2", name="sinn2")
        mask01 = pers.tile([P, P], bf, tag="m01", name="m01")
        qT = [pers.tile([P, S], bf, tag=f"qT{m}", name=f"qT{m}") for m in range(2)]
        kTt = [pers.tile([P, S], bf, tag=f"kT{m}", name=f"kT{m}") for m in range(2)]
        qR = [pers.tile([P, S], bf, tag=f"qR{m}", name=f"qR{m}") for m in range(2)]
        kR = [pers.tile([P, S], bf, tag=f"kR{m}", name=f"kR{m}") for m in range(2)]
        vsb = [pers.tile([P, 4 * 65], bf, tag=f"v{j}", name=f"v{j}") for j in range(16)]
        aT = [pers.tile([P, S], bf, tag=f"aT{m}", name=f"aT{m}") for m in range(2)]
        ones1 = pers.tile([1, 64], bf, tag="ones1", name="ones1")

        # ---- input DMAs ----
        for k in range(8):
            nc.sync.dma_start(out=wq[k][:], in_=wq_d[P * k : P * (k + 1), :])
            nc.sync.dma_start(out=wk[k][:], in_=wk_d[P * k : P * (k + 1), :])
            nc.sync.dma_start(out=wv[k][:], in_=wv_d[P * k : P * (k + 1), :])
            nc.sync.dma_start(out=xt[k][:], in_=xt_d[P * k : P * (k + 1), :])
        for m in range(2):
            nc.sync.dma_start(out=wo[m][:], in_=wo_d[P * m : P * (m + 1), :])
        nc.sync.dma_start(out=cos2[:], in_=cos_d[:, :])
        nc.sync.dma_start(out=sinn2[:], in_=sin_d[:, :])
        nc.sync.dma_start(out=mask01[:], in_=msk_d[:, :])

        nc.vector.memset(ones1[:], 1.0)
        # preset V tiles to 1.0 so the per-head 65th (ones) column survives
        for j in range(16):
            nc.gpsimd.memset(vsb[j][:], 1.0)

        # HAM warm-up: dense dummy matmuls while the input DMAs stream in
        wup = pers.tile([P, 512], bf, tag="wup", name="wup")
        nc.vector.memset(wup[:], 0.25)
        wps = ps512.tile([P, 512], f32, tag="ps512", name="ps")
        for i in range(24):
            nc.tensor.matmul(
                wps[:], lhsT=wup[:, 0:P], rhs=wup[:], start=(i == 0), stop=(i == 23)
            )

        # ---- emission helpers (each emits a self-contained chunk) ----
        def qk_chunk(wt, dstT, m, n, evict_dve):
            ps = ps512.tile([P, 512], f32, tag="ps512", name="ps")
            for k in range(8):
                nc.tensor.matmul(
                    ps[:],
                    lhsT=wt[k][:, P * m : P * (m + 1)],
                    rhs=xt[k][:, 512 * n : 512 * (n + 1)],
                    start=(k == 0),
                    stop=(k == 7),
                )
            if evict_dve:
                nc.vector.tensor_copy(
                    out=dstT[m][:, 512 * n : 512 * (n + 1)], in_=ps[:]
                )
            else:
                nc.scalar.copy(dstT[m][:, 512 * n : 512 * (n + 1)], ps[:])

        def v_chunk(st):
            ps = ps512.tile([P, 512], f32, tag="ps512", name="ps")
            for k in range(8):
                nc.tensor.matmul(
                    ps[:, 0:E],
                    lhsT=xt[k][:, P * st : P * (st + 1)],
                    rhs=wv[k][:],
                    start=(k == 0),
                    stop=(k == 7),
                )
            nc.vector.tensor_copy(
                out=vsb[st][:].rearrange("p (h x) -> p h x", x=65)[:, :, 0:64],
                in_=ps[:, 0:E].rearrange("p (h x) -> p h x", x=64),
            )

        rope_state = {}

        def rope_op(src, dst, m, i):
            # i in 0..5: 4 rotated-half sin multiplies, cos multiply, add
            key = (id(src), m)
            if i == 0:
                rope_state[key] = (
                    rtmp_pool.tile([P, S], bf, tag="rtmp", name="rtmp"),
                    rtmp_pool.tile([P, S], bf, tag="rtmp2", name="rtmp2"),
                )
            tmp, tmp2 = rope_state[key]
            if i < 4:
                o = 32 * i
                so = o + 32 if i % 2 == 0 else o - 32
                eng = nc.vector if i % 2 == 0 else nc.gpsimd
                eng.tensor_mul(
                    tmp[o : o + 32, :], src[m][so : so + 32, :], sinn2[so : so + 32, :]
                )
            elif i == 4:
                nc.vector.tensor_mul(tmp2[:], src[m][:], cos2[:])
            else:
                nc.vector.tensor_add(dst[m][:], tmp2[:], tmp[:])

        # ---- prologue: first-tile projections + RoPE, first V chunks ----
        for n in range(4):
            qk_chunk(wk, kTt, 0, n, evict_dve=False)
        for n in range(4):
            qk_chunk(wq, qT, 0, n, evict_dve=False)
            rope_op(kTt, kR, 0, n)  # k-rope rides under the Q projection
        rope_op(kTt, kR, 0, 4)
        rope_op(kTt, kR, 0, 5)
        for i in range(6):
            rope_op(qT, qR, 0, i)
        for st in range(8):
            v_chunk(st)

        # filler schedules: (head, kj) -> list of emitters.  PSUM-using fillers
        # only from kj >= 6 (all four at[] banks are live before the first
        # normalization frees one).
        psum_fillers = (
            [lambda st=st: v_chunk(st) for st in range(8, 16)]
            + [lambda n=n: qk_chunk(wq, qT, 1, n, evict_dve=True) for n in range(4)]
            + [lambda n=n: qk_chunk(wk, kTt, 1, n, evict_dve=True) for n in range(4)]
        )
        dve_fillers = [
            (lambda i=i: rope_op(qT, qR, 1, i)) for i in range(6)
        ] + [(lambda i=i: rope_op(kTt, kR, 1, i)) for i in range(6)]

        def fillers_for(h, kj):
            # V(st) fillers lead their first consumer (head 0, kj=st) by 2
            out = []
            if h == 0 and kj >= 6 and psum_fillers:
                out.append(psum_fillers.pop(0))  # V8..15 then Qm1 n0,n1
            elif h == 1 and 6 <= kj and psum_fillers:
                out.append(psum_fillers.pop(0))  # Qm1 n2,n3 + Km1 n0..3
            if h == 1 and kj >= 8 and dve_fillers:
                out.append(dve_fillers.pop(0))
                out.append(dve_fillers.pop(0))
            return out

        # ---- attention ----
        pending = []

        def emit_norm(m_, r_, at_, qc_):
            rc = rc_pool.tile([1, 512], bf, tag="rc", name="rc")
            with nc.allow_low_precision(reason="softmax denom recip bf16"):
                nc.vector.reciprocal(rc[:], at_[qc_][64:65, :])
            rbp = sc_ps.tile([64, 512], f32, tag="scB", name="rbp")
            nc.tensor.matmul(rbp[:], lhsT=ones1[:], rhs=rc[:], start=True, stop=True)
            rb = rb_pool.tile([64, 512], f32, tag="rb", name="rb")
            nc.vector.tensor_copy(out=rb[:], in_=rbp[:])
            nc.vector.tensor_mul(
                aT[m_][r_ : r_ + 64, 512 * qc_ : 512 * (qc_ + 1)],
                at_[qc_][0:64, :],
                rb[:],
            )

        for h in range(4):
            m = h // 2
            r = 64 * (h % 2)
            at = [ps512.tile([P, 512], f32, tag="ps512", name="ps") for _ in range(4)]
            for kj in range(16):
                qs0 = P * kj
                nq = S - qs0
                for c1 in range((nq + 1023) // 1024):
                    cs = qs0 + 1024 * c1
                    w = min(1024, S - cs)
                    tag = (
                        "scA"
                        if (kj < 10 and c1 == 0) or (kj >= 10 and kj % 2 == 0)
                        else "scB"
                    )
                    sc = sc_ps.tile([P, 1024], f32, tag=tag, name="sc")
                    for c2 in range((w + 511) // 512):
                        w2 = min(512, w - 512 * c2)
                        nc.tensor.matmul(
                            sc[:, 512 * c2 : 512 * c2 + w2],
                            lhsT=kR[m][r : r + 64, qs0 : qs0 + P],
                            rhs=qR[m][r : r + 64, cs + 512 * c2 : cs + 512 * c2 + w2],
                            start=True,
                            stop=True,
                        )
                    ex = ex_pool.tile([P, 1024], bf, tag="ex", name="ex")
                    nc.scalar.activation(ex[:, 0:w], sc[:, 0:w], EXP, scale=SCALE)
                    if c1 == 0:
                        nc.gpsimd.tensor_mul(ex[:, 0:P], ex[:, 0:P], mask01[:])
                    # one attention MM per overlapped 512-wide query chunk
                    for qc in range(cs // 512, (cs + w + 511) // 512):
                        lo = max(cs, 512 * qc)
                        hi = min(cs + w, 512 * (qc + 1))
                        if hi <= lo:
                            continue
                        nc.tensor.matmul(
                            at[qc][0:65, lo - 512 * qc : hi - 512 * qc],
                            lhsT=vsb[kj][:, 65 * h : 65 * h + 65],
                            rhs=ex[:, lo - cs : hi - cs],
                            start=(kj == 0),
                            stop=(kj == 4 * qc + 3),
                        )
                    while pending:
                        emit_norm(*pending.pop(0))
                if kj >= 3 and (kj - 3) % 4 == 0:
                    pending.append((m, r, at, (kj - 3) // 4))
                for f in fillers_for(h, kj):
                    f()

        while pending:
            emit_norm(*pending.pop(0))

        # ---- partial out-projection ----
        for st in range(16):
            for dc in range(2):
                po = ps512.tile([P, 512], f32, tag="ps512", name="ps")
                for kk in range(2):
                    nc.tensor.matmul(
                        po[:],
                        lhsT=aT[kk][:, P * st : P * (st + 1)],
                        rhs=wo[kk][:, 512 * dc : 512 * (dc + 1)],
                        start=(kk == 0),
                        stop=(kk == 1),
                    )
                ov = oev_pool.tile([P, 512], f32, tag="oev", name="ov")
                nc.scalar.copy(ov[:], po[:])
                nc.sync.dma_start(
                    out=out_d[P * st : P * (st + 1), 512 * dc : 512 * (dc + 1)],
                    in_=ov[:],
                )

    nc.finalize()
    return nc


def _tables():
    inv = 1.0 / (10000.0 ** (np.arange(0, HD, 2, dtype=np.float64) / HD))
    fr = np.outer(np.arange(S, dtype=np.float64), inv)  # [S, 32]
    emb = np.concatenate([fr, fr], axis=1)  # [S, 64]
    cosT = np.cos(emb).T.astype(np.float32)  # [64, S]
    sinT = np.sin(emb).T.astype(np.float32)
    # indexed by the rot SOURCE partition: row so holds the sin factor that
    # multiplies q[so] when it lands at the rotated destination partition
    sinn = np.concatenate([sinT[32:64], -sinT[0:32]], axis=0)
    cos2 = np.concatenate([cosT, cosT], axis=0).astype(BF16)  # [128, S]
    sinn2 = np.concatenate([sinn, sinn], axis=0).astype(BF16)
    # mask01[j, i] = 1 where key j <= query i (valid): upper triangle incl. diag
    mask01 = np.triu(np.ones((P, P), dtype=np.float32)).astype(BF16)
    return cos2, sinn2, mask01


def kernel(embeds, Wq, Wk, Wv, Wo, bo):
    embeds = np.asarray(embeds, dtype=np.float32)
    Wq = np.asarray(Wq, dtype=np.float32)
    Wk = np.asarray(Wk, dtype=np.float32)
    Wv = np.asarray(Wv, dtype=np.float32)
    Wo = np.asarray(Wo, dtype=np.float32)
    bo = np.asarray(bo, dtype=np.float32)

    if "nc" not in _CACHE:
        _CACHE["nc"] = build_nc()
    nc = _CACHE["nc"]

    xtb = [np.ascontiguousarray(embeds[b].T).astype(BF16) for b in range(2)]
    wqb = Wq.astype(BF16)
    wkb = Wk.astype(BF16)
    wvb = Wv.astype(BF16)
    wob = Wo.astype(BF16)

    cos2, sinn2, mask01 = _tables()
    in_maps = []
    for c in range(N_CORES):
        b, g = c // 4, c % 4
        rows = slice(E * g, E * (g + 1))
        in_maps.append(
            {
                "xt": xtb[b],
                "wq_t": np.ascontiguousarray(wqb[rows].T),
                "wk_t": np.ascontiguousarray(wkb[rows].T),
                "wv_t": np.ascontiguousarray(wvb[rows].T),
                "wo_t": np.ascontiguousarray(wob[:, rows].T),
                "cos2": cos2,
                "sinn2": sinn2,
                "mask01": mask01,
            }
        )

    trace = bool(int(os.environ.get("BASS_KERNEL_TRACE", "0")))
    res = run_bass_kernel_spmd(nc, in_maps, list(range(N_CORES)), trace=trace)
    _CACHE["last_results"] = res

    out = np.zeros((2, S, D), dtype=np.float32)
    for c in range(N_CORES):
        out[c // 4] += res.results[c]["out_p"]
    out += bo[None, None, :]
    return out
